# revision 3
# baseline (speedup 1.0000x reference)
"""Trainium2 Bass kernel for nn_LinkPredictor.

Reference computation (B=4, N=256, T=16, F=128, H=256):
    h = mean_T(nodefeat)                      # [B,N,F]
    a = h @ W1[:, :F].T                       # [B,N,H]
    c = h @ W1[:, F:].T                       # [B,N,H]
    logits[b,i,j] = W2[0] . relu(a[b,i] + c[b,j] + b1) + b2   # [B,N,N]

Sharding: 8 cores; core k handles batch b=k//2, i-half k%2 (128 i-rows x
256 j-cols of one batch's NxN grid).  Each core only needs nodefeat[b].

v2 changes vs baseline (59.1us):
  - nf DMA split across BOTH HWDGE queues (sync + scalar) -> ~2x faster
    input landing; weights packed into 2 small params (w2b diagonal
    matrix replaced by a [128,2,63] sliding-window tensor).
  - PE pre-warmed with dummy N=8 matmuls during the DMA wait so the HAM
    clock-gate releases (1.2->2.4GHz) before the pairwise phase starts.
  - W1 matmuls in bf16 (hT copied to bf16 once) - smaller DMA, 1cyc/col.
  - act tile split VE/SE at measured rates (196ns vs 398ns spacing).
  - drains (+b2) on ScalarE from PSUM; output DMAs alternate queues.
"""

import os
import sys

import numpy as np

_B, _N, _T, _F, _H = 4, 256, 16, 128, 256
_NCORES = 8
_NWARM = 64  # dummy PE-warm matmuls

_CACHE = {}


def _ensure_paths():
    for p in (
        "/root/.axon_site",
        "/root/.axon_site/_ro/trn_rl_repo",
        "/root/.axon_site/_ro/pypackages",
        "/opt/trn_rl_repo",
    ):
        if os.path.isdir(p) and p not in sys.path:
            sys.path.append(p)


# wsmall free-dim layout (f32), 16B-aligned scalar columns
_WS_SMAT = 0       # [0:8)    smat f32 (cast to bf16 on device)
_WS_W2P = 8        # [8:134)  w2pad [2, 63] f32 (cast to bf16 on device)
_WS_B1T0 = 136     # b1t for t=0 (16B aligned: 136*4=544)
_WS_B1T1 = 140     # b1t for t=1
_WS_B2C = 144      # b2
_WS_W = 148


def build_nc():
    """Build the per-core Bass program (same program for all 8 cores)."""
    _ensure_paths()
    import concourse.mybir as mybir
    import concourse.tile as tile
    from concourse import bacc

    f32 = mybir.dt.float32
    bf16 = mybir.dt.bfloat16
    Alu = mybir.AluOpType
    Act = mybir.ActivationFunctionType

    nc = bacc.Bacc("TRN2", target_bir_lowering=False, debug=False)

    nf = nc.declare_dram_parameter("nf", [128, 32, 128], bf16, isOutput=False)
    wsmall = nc.declare_dram_parameter("wsmall", [128, _WS_W], f32, isOutput=False)
    w1b = nc.declare_dram_parameter("w1b", [128, 4, 128], bf16, isOutput=False)
    outd = nc.declare_dram_parameter("out", [4, 32, 256], f32, isOutput=True)

    with tile.TileContext(nc) as tc:
        with (
            tc.tile_pool(name="const", bufs=1) as constp,
            tc.tile_pool(name="data", bufs=1) as datap,
            tc.tile_pool(name="act", bufs=18) as actp,
            tc.tile_pool(name="ph", bufs=1, space="PSUM") as php,
            tc.tile_pool(name="pc", bufs=2, space="PSUM") as pcp,
            tc.tile_pool(name="pl", bufs=2, space="PSUM") as plp,
            tc.tile_pool(name="pw", bufs=1, space="PSUM") as pwp,
        ):
            # ---- DMAs: two parallel HWDGE queues (sync + scalar) ----
            wsmall_sb = constp.tile([128, _WS_W], f32, tag="wsmall")
            nc.sync.dma_start(out=wsmall_sb[:], in_=wsmall[:])
            nf_sb = constp.tile([128, 32, 128], bf16, tag="nf")
            # sync: octets 0-7, 16-23;  scalar: octets 8-15, 24-31, w1b
            nc.sync.dma_start(out=nf_sb[:, 0:8, :], in_=nf[:, 0:8, :])
            nc.scalar.dma_start(out=nf_sb[:, 8:16, :], in_=nf[:, 8:16, :])
            nc.sync.dma_start(out=nf_sb[:, 16:24, :], in_=nf[:, 16:24, :])
            nc.scalar.dma_start(out=nf_sb[:, 24:32, :], in_=nf[:, 24:32, :])
            w1b_sb = constp.tile([128, 4, 128], bf16, tag="w1b")
            nc.scalar.dma_start(out=w1b_sb[:], in_=w1b[:])

            # ---- PE warm-up: dummy matmuls during the DMA wait ----
            scratch = constp.tile([128, 8], bf16, tag="scratch")
            nc.vector.memset(scratch[:], 0.0)
            pwarm = pwp.tile([8, 8], f32, tag="pwarm")
            for _ in range(_NWARM):
                nc.tensor.matmul(
                    pwarm[:], lhsT=scratch[:], rhs=scratch[:], start=True, stop=True
                )

            # on-device casts of smat / w2pad to bf16
            smat_sb = constp.tile([128, 8], bf16, tag="smat")
            nc.vector.tensor_copy(smat_sb[:], wsmall_sb[:, _WS_SMAT : _WS_SMAT + 8])
            w2pb = constp.tile([128, 2, 63], bf16, tag="w2pb")
            nc.vector.tensor_copy(
                w2pb[:], wsmall_sb[:, _WS_W2P : _WS_W2P + 126]
            )

            # ---- hT[f, j] via 32 per-octet matmuls against S ----
            ph = php.tile([128, 256], f32, tag="ph")
            for o in range(32):
                nc.tensor.matmul(
                    ph[:, 8 * o : 8 * o + 8],
                    lhsT=nf_sb[:, o, :],
                    rhs=smat_sb[:],
                    start=True,
                    stop=True,
                )
            hTb = datap.tile([128, 256], bf16, tag="hTb")
            nc.vector.tensor_copy(hTb[:], ph[:])

            # ---- cT (bf16) and aTb4 (fp32, b1-folded, x4-replicated) ----
            cT = [
                datap.tile([128, 256], bf16, tag=f"cT{t}", name=f"cT{t}")
                for t in range(2)
            ]
            aTb4 = [
                datap.tile([128, 128, 4], f32, tag=f"aTb4{t}", name=f"aTb4{t}")
                for t in range(2)
            ]
            for t in range(2):
                pc = pcp.tile([128, 256], f32, tag="pc")
                nc.tensor.matmul(
                    pc[:], lhsT=w1b_sb[:, 2 + t, :], rhs=hTb[:], start=True, stop=True
                )
                nc.scalar.copy(cT[t][:], pc[:])
                pa = pcp.tile([128, 128], f32, tag="pa")
                nc.tensor.matmul(
                    pa[:], lhsT=w1b_sb[:, t, :], rhs=hTb[:, 0:128], start=True, stop=True
                )
                nc.vector.tensor_scalar(
                    aTb4[t][:, :, :],
                    pa[:].broadcast_to([128, 128, 4]),
                    wsmall_sb[:, _WS_B1T0 + 4 * t : _WS_B1T0 + 4 * t + 1],
                    None,
                    Alu.add,
                )

            # ---- pairwise: act tiles on VE/SE, reduction on PE ----
            # stationary for (t, r): w2pb[:, t, 31-r : 63-r]  (col r = w2_t)
            for g in range(4):
                pl = plp.tile([32, 256], f32, tag="pl", name=f"pl{g}")
                for t in range(2):
                    for r in range(32):
                        i = 32 * g + r
                        idx = 2 * i + t
                        a_col = aTb4[t][:, i, 0:1]
                        if idx % 3 == 1:
                            asb = actp.tile([128, 256], bf16, tag="acts")
                            nc.scalar.activation(
                                asb[:], cT[t][:], Act.Relu, bias=a_col
                            )
                            mv = asb
                        else:
                            av = actp.tile([128, 256], bf16, tag="actv")
                            nc.vector.tensor_scalar(
                                av[:], cT[t][:], a_col, 0.0, Alu.add, Alu.max
                            )
                            mv = av
                        nc.tensor.matmul(
                            pl[:, :],
                            lhsT=w2pb[:, t, 31 - r : 63 - r],
                            rhs=mv[:],
                            start=(t == 0 and r == 0),
                            stop=(t == 1 and r == 31),
                        )
                osb = datap.tile([32, 256], f32, tag=f"osb{g}", name=f"osb{g}")
                nc.scalar.activation(
                    osb[:], pl[:, :], Act.Identity, bias=wsmall_sb[0:32, _WS_B2C : _WS_B2C + 1]
                )
                if g % 2 == 0:
                    nc.sync.dma_start(out=outd[g], in_=osb[:])
                else:
                    nc.scalar.dma_start(out=outd[g], in_=osb[:])

    nc.compile()
    return nc


def make_in_maps(nodefeat, W1, b1, W2, b2):
    """Host-side sharding/layout prep."""
    import ml_dtypes

    bf16 = ml_dtypes.bfloat16
    nodefeat = np.asarray(nodefeat, dtype=np.float32)
    W1 = np.asarray(W1, dtype=np.float32)
    b1 = np.asarray(b1, dtype=np.float32)
    W2 = np.asarray(W2, dtype=np.float32)
    b2 = np.asarray(b2, dtype=np.float32)

    # wsmall pack (f32): smat, w2pad, b1t (16B-aligned cols), b2
    wsmall = np.zeros((128, _WS_W), dtype=np.float32)
    wsmall[:, _WS_SMAT : _WS_SMAT + 8] = np.repeat(
        np.eye(8, dtype=np.float32), 16, axis=0
    ) / 16.0
    w2r = W2[0].reshape(2, 128)  # [t, p]
    w2pad = np.zeros((128, 2, 63), dtype=np.float32)
    w2pad[:, :, 31] = w2r.T
    wsmall[:, _WS_W2P : _WS_W2P + 126] = w2pad.reshape(128, 126)
    b1t = b1.reshape(2, 128).T  # [p, t]
    wsmall[:, _WS_B1T0] = b1t[:, 0]
    wsmall[:, _WS_B1T1] = b1t[:, 1]
    wsmall[:, _WS_B2C] = b2[0]

    # w1b pack (bf16): [p=f, {a0,a1,c0,c1}, h_t]
    W1a, W1c = W1[:, :_F], W1[:, _F:]
    w1b = np.stack(
        [W1a[:128].T, W1a[128:].T, W1c[:128].T, W1c[128:].T], axis=1
    ).astype(bf16)
    w1b = np.ascontiguousarray(w1b)

    in_maps = []
    for k in range(_NCORES):
        b, ih = divmod(k, 2)
        nf_b = nodefeat[b]  # [256, 16, 128]
        if ih:
            nf_b = np.concatenate([nf_b[128:], nf_b[:128]], axis=0)
        # [256,16,128] -> [32 oct, (j8,t16)=128, 128 f] -> [128, 32, 128]
        nf_dev = np.ascontiguousarray(
            nf_b.reshape(32, 128, 128).transpose(1, 0, 2).astype(bf16)
        )
        in_maps.append({"nf": nf_dev, "wsmall": wsmall, "w1b": w1b})
    return in_maps


def assemble_output(results):
    out = np.empty((_B, _N, _N), dtype=np.float32)
    for k in range(_NCORES):
        b, ih = divmod(k, 2)
        r = results[k]["out"].reshape(128, 256)  # [i, j] (j core-local order)
        if ih:
            r = np.concatenate([r[:, 128:], r[:, :128]], axis=1)
        out[b, ih * 128 : (ih + 1) * 128, :] = r
    return out


def _get_nc():
    if "nc" not in _CACHE:
        _CACHE["nc"] = build_nc()
    return _CACHE["nc"]


def kernel(nodefeat, W1, b1, W2, b2):
    _ensure_paths()
    from concourse.bass_utils import run_bass_kernel_spmd

    nc = _get_nc()
    in_maps = make_in_maps(nodefeat, W1, b1, W2, b2)
    res = run_bass_kernel_spmd(nc, in_maps, list(range(_NCORES)))
    return assemble_output(res.results)
